# revision 5
# baseline (speedup 1.0000x reference)
"""Trainium2 Bass kernel for the 3-modality 9-branch cross-attention module.

Sharding: data-parallel over batch (16) across 8 NeuronCores (2 per core).
Per core, per batch element:
  stage 1: X^T via PE transpose; Q^T,K^T (head-major, [768, 897]) and
           V'' (row-major with interleaved ones columns) via fp32r matmuls.
  stage 2: per head / per kv-modality: S^T = K_h @ Q_h^T (all 3 query
           modalities packed along the 897-wide free dim), E^T = exp(S^T/8)
           (no max subtraction needed: |s|/8 < ~2.5), then ctx'^T = V''^T E^T
           which yields both the unnormalized context rows and 64 replicated
           denominator rows (ones columns of V''); normalize with DVE
           reciprocal+mul and accumulate C^T across kv-modalities.
  stage 2b: img self-attention probabilities recomputed row-major
           (S = Q_h K_h^T, exp with accum_out row-sums, tensor_scalar
           normalize) and DMA'd out as the `weights` output.
  stage 3: OUT = C^T.T @ Wo + bo (bias via K=1 ones-row matmul).

PSUM matmul destinations are laid out as [128, nregions, 512] so every
matmul's output region sits inside a single 2 KiB PSUM bank.
fp32r matmuls require an even moving-column count, so odd widths are
padded up by one column (zeroed where the pad would feed exp).
"""
import os
import sys

for _p in ("/opt/trn_rl_repo", "/root/.axon_site/_ro/trn_rl_repo"):
    if os.path.isdir(_p) and _p not in sys.path:
        sys.path.insert(0, _p)

import numpy as np
import concourse.bass as bass
import concourse.mybir as mybir
import concourse.tile as tile
from concourse import bacc, bass_utils
from concourse.masks import make_identity

FP32 = mybir.dt.float32
FP32R = mybir.dt.float32r
AF = mybir.ActivationFunctionType

B, D, H, DH = 16, 768, 12, 64
NI, NT, NCL = 577, 256, 64
NQ = NI + NT + NCL          # 897
NQE = NQ + 1                # padded even width for fp32r matmuls
SCALE = 1.0 / 8.0
NCORES = 8
BPC = B // NCORES           # 2
DC = D // 128               # 6

# (weight suffix, seq len, offset in the 897-concat, input tensor name)
MODS = [("", NI, 0, "hidden_states"),
        ("_t", NT, NI, "text"),
        ("_c", NCL, NI + NT, "clinical")]

WNAMES = ["Wq", "Wk", "Wv", "Wo", "Wq_t", "Wk_t", "Wv_t", "Wo_t",
          "Wq_c", "Wk_c", "Wv_c", "Wo_c"]


def _chunks(n):
    out, o = [], 0
    while o < n:
        p = min(128, n - o)
        out.append((o, p))
        o += p
    return out


def _fsplits(n):
    """Even-size splits of a (padded-up if odd) free dim into pieces <=512,
    each >=256 when possible: fp32r matmuls need an even number of moving
    columns and run 4x slower below 256 of them. Each piece maps to its own
    512-wide PSUM bank region."""
    m = n + (n & 1)
    if m <= 512:
        return [(0, m)]
    a = (m // 4) * 2
    return [(0, m - a), (m - a, a)]


# global k-chunk list: (mod index, global col offset in 897, rows p)
KCHUNKS = []
KC_OF_MOD = {0: [], 1: [], 2: []}
for _mi, (_sfx, _n, _off, _nm) in enumerate(MODS):
    for (_o, _p) in _chunks(_n):
        KC_OF_MOD[_mi].append(len(KCHUNKS))
        KCHUNKS.append((_mi, _off + _o, _p))
NKC = len(KCHUNKS)          # 8

FS_NQ = _fsplits(NQ)        # [(0,450),(450,448)]
FS_D = _fsplits(D)          # [(0,384),(384,384)]
FS_NI = _fsplits(NI)        # [(0,290),(290,288)]


def build():
    nc = bacc.Bacc("TRN2", target_bir_lowering=False, debug=False,
                   num_devices=NCORES)

    x_in = {}
    for sfx, n, off, nm in MODS:
        x_in[nm] = nc.dram_tensor(nm, [BPC, n, D], FP32, kind="ExternalInput")
    W = {nm: nc.dram_tensor(nm, [D, D], FP32, kind="ExternalInput")
         for nm in WNAMES}
    Bv = {}
    for nm in WNAMES:
        bn = "b" + nm[1:]
        Bv[bn] = nc.dram_tensor(bn, [D], FP32, kind="ExternalInput")

    out_d = {
        "": nc.dram_tensor("out_img", [BPC, NI, D], FP32, kind="ExternalOutput"),
        "_t": nc.dram_tensor("out_txt", [BPC, NT, D], FP32, kind="ExternalOutput"),
        "_c": nc.dram_tensor("out_cln", [BPC, NCL, D], FP32, kind="ExternalOutput"),
    }
    attn_w = nc.dram_tensor("attn_w", [BPC, H, NI, NI], FP32,
                            kind="ExternalOutput")
    dbg = {}
    if os.environ.get("KERNEL_DEBUG") == "1":
        dbg["xT"] = nc.dram_tensor("dbg_xT", [128, DC, NQ], FP32, kind="ExternalOutput")
        dbg["qt"] = nc.dram_tensor("dbg_qt", [128, DC, NQE], FP32, kind="ExternalOutput")
        dbg["kt"] = nc.dram_tensor("dbg_kt", [128, DC, NQE], FP32, kind="ExternalOutput")
        dbg["vpp"] = nc.dram_tensor("dbg_vpp", [128, NKC, H * 128], FP32, kind="ExternalOutput")
        dbg["ct"] = nc.dram_tensor("dbg_ct", [128, DC, NQE], FP32, kind="ExternalOutput")

    with tile.TileContext(nc) as tc:
        with tc.tile_pool(name="const", bufs=1) as cpool:
            ident = cpool.tile([128, 128], FP32, name="ident")
            make_identity(nc, ident)
            ones_row = cpool.tile([1, 128], FP32R, name="ones_row")
            nc.vector.memset(ones_row.bitcast(FP32), 1.0)
            # per-partition bias tiles for Q/K (bias indexed by output dim,
            # which lands on partitions), row-layout biases for V / O.
            bqk = {}
            brow = {}
            for sfx, _, _, _ in MODS:
                for kind in ("q", "k"):
                    t = cpool.tile([128, DC], FP32, name=f"b{kind}{sfx}_p")
                    nc.sync.dma_start(
                        out=t, in_=Bv[f"b{kind}{sfx}"][:].rearrange(
                            "(c p) -> p c", p=128))
                    bqk[(kind, sfx)] = t
                for kind in ("v", "o"):
                    t = cpool.tile([1, D], FP32R, name=f"b{kind}{sfx}_r")
                    nc.sync.dma_start(
                        out=t, in_=Bv[f"b{kind}{sfx}"][:].rearrange(
                            "(a d) -> a d", a=1).bitcast(FP32R))
                    brow[(kind, sfx)] = t

            for b in range(BPC):
                _one_batch(nc, tc, b, x_in, W, out_d, attn_w,
                           ident, ones_row, bqk, brow,
                           dbg if b == 0 else {})
    nc.compile()
    return nc


def _one_batch(nc, tc, b, x_in, W, out_d, attn_w, ident, ones_row, bqk, brow,
               dbg):
    from contextlib import ExitStack
    with ExitStack() as bstk:
        pb = bstk.enter_context(tc.tile_pool(name=f"pb{b}", bufs=1))
        qt = pb.tile([128, DC, NQE], FP32R, name=f"qt{b}")
        kt = pb.tile([128, DC, NQE], FP32R, name=f"kt{b}")
        vpp = pb.tile([128, NKC, H * 128], FP32R, name=f"vpp{b}")
        ct = pb.tile([128, DC, NQE], FP32R, name=f"ct{b}")
        # zero the pad column so exp(pad)=1 stays finite downstream
        nc.vector.memset(qt[:, :, NQ:NQE].bitcast(FP32), 0.0)
        nc.vector.memset(kt[:, :, NQ:NQE].bitcast(FP32), 0.0)

        # ---------------- stage 1: X^T + projections ----------------
        with tc.tile_pool(name=f"s1sb{b}", bufs=1) as s1sb, \
             tc.tile_pool(name=f"s1w{b}", bufs=2) as s1w, \
             tc.tile_pool(name=f"s1x{b}", bufs=3) as s1x, \
             tc.tile_pool(name=f"ps_tr{b}", bufs=3, space="PSUM") as ps_tr, \
             tc.tile_pool(name=f"ps1{b}", bufs=2, space="PSUM") as ps1:
            xT = s1sb.tile([128, DC, NQ], FP32R, name=f"xT{b}")

            # ones columns of V'': cols [64:192] of every 256-wide double-head
            # block (even head ctx|ones, odd head ones|ctx ordering)
            vview = vpp[:, :, :].rearrange("p j (g x) -> p j g x", x=256)
            nc.vector.memset(vview[:, :, :, 64:192].bitcast(FP32), 1.0)

            for mi, (sfx, n, qoff, nm) in enumerate(MODS):
                xd = x_in[nm]
                # X^T via PE transpose
                for (o, p) in _chunks(n):
                    xrow = s1x.tile([128, D], FP32, name="xrow", tag="xrow")
                    nc.sync.dma_start(out=xrow[:p, :], in_=xd[b, o:o + p, :])
                    for dci in range(DC):
                        trp = ps_tr.tile([128, 128], FP32, name="trp", tag="trp")
                        nc.tensor.transpose(
                            trp[:, :p], xrow[:p, dci * 128:(dci + 1) * 128],
                            ident[:p, :p])
                        nc.vector.tensor_copy(
                            xT[:, dci, qoff + o:qoff + o + p], trp[:, :p])

                # Q^T and K^T (head-major): out rows = output dim
                fs_n = _fsplits(n)
                for kind in ("q", "k"):
                    wt = s1w.tile([128, DC, D], FP32R, name="wt", tag="wt")
                    nc.sync.dma_start(
                        out=wt, in_=W[f"W{kind}{sfx}"][:, :].rearrange(
                            "(c p) o -> p c o", p=128).bitcast(FP32R))
                    dst = qt if kind == "q" else kt
                    for doc in range(DC):
                        pq = ps1.tile([128, 2, 512], FP32, name="pq", tag="ps1")
                        for ri, (fo, fl) in enumerate(fs_n):
                            for dci in range(DC):
                                nc.tensor.matmul(
                                    pq[:, ri, 0:fl],
                                    wt[:, dci, doc * 128:(doc + 1) * 128],
                                    xT[:, dci, qoff + fo:qoff + fo + fl],
                                    start=(dci == 0), stop=(dci == DC - 1))
                        for ri, (fo, fl) in enumerate(fs_n):
                            flv = min(fl, n - fo)
                            nc.vector.tensor_scalar_add(
                                dst[:, doc, qoff + fo:qoff + fo + flv],
                                pq[:, ri, 0:flv],
                                bqk[(kind, sfx)][:, doc:doc + 1])

                # V (row-major) into per-head 128-wide blocks of vpp
                wt = s1w.tile([128, DC, D], FP32R, name="wt", tag="wt")
                nc.sync.dma_start(
                    out=wt, in_=W[f"Wv{sfx}"][:, :].rearrange(
                        "(c p) o -> p c o", p=128).bitcast(FP32R))
                for ci, (o, p) in enumerate(_chunks(n)):
                    j = KC_OF_MOD[mi][ci]
                    pv = ps1.tile([128, 2, 512], FP32, name="pv", tag="ps1")
                    for ri, (fo, fl) in enumerate(FS_D):
                        for dci in range(DC):
                            nc.tensor.matmul(
                                pv[:p, ri, 0:fl],
                                xT[:, dci, qoff + o:qoff + o + p],
                                wt[:, dci, fo:fo + fl],
                                start=(dci == 0), stop=False)
                        nc.tensor.matmul(
                            pv[:p, ri, 0:fl], ones_row[0:1, 0:p],
                            brow[("v", sfx)][0:1, fo:fo + fl],
                            start=False, stop=True)
                    # scatter d-cols into per-head blocks:
                    # region 0 holds d 0:384 (head-pairs g=0..2),
                    # region 1 holds d 384:768 (g=3..5)
                    dvv = vpp[:p, j, :].rearrange("p (g x) -> p g x", x=256)
                    for ri in range(2):
                        pvv = pv[:p, ri, :].rearrange(
                            "p (g x) -> p g x", x=128)
                        g0 = ri * 3
                        nc.vector.tensor_copy(dvv[:, g0:g0 + 3, 0:64],
                                              pvv[:, 0:3, 0:64])
                        nc.vector.tensor_copy(dvv[:, g0:g0 + 3, 192:256],
                                              pvv[:, 0:3, 64:128])
            if dbg:
                nc.sync.dma_start(out=dbg["xT"][:, :, :], in_=xT.bitcast(FP32))
                nc.sync.dma_start(out=dbg["qt"][:, :, :], in_=qt.bitcast(FP32))
                nc.sync.dma_start(out=dbg["kt"][:, :, :], in_=kt.bitcast(FP32))
                nc.sync.dma_start(out=dbg["vpp"][:, :, :], in_=vpp.bitcast(FP32))

        # ---------------- stage 2: attention (col-major) ----------------
        with tc.tile_pool(name=f"s2e{b}", bufs=3) as s2e, \
             tc.tile_pool(name=f"s2r{b}", bufs=2) as s2r, \
             tc.tile_pool(name=f"s2t{b}", bufs=2) as s2t, \
             tc.tile_pool(name=f"ps_s{b}", bufs=2, space="PSUM") as ps_s, \
             tc.tile_pool(name=f"ps_c{b}", bufs=2, space="PSUM") as ps_c:
            for h in range(H):
                po, pr = (h % 2) * 64, h // 2
                co = po            # ctx rows offset in ctx' psum
                do_ = 64 - po      # denominator rows offset
                for ki in range(3):
                    kjs = KC_OF_MOD[ki]
                    ctx = ps_c.tile([128, 2, 512], FP32, name="ctx", tag="ctx")
                    for jj, j in enumerate(kjs):
                        _, koff, p = KCHUNKS[j]
                        s_ps = ps_s.tile([128, 2, 512], FP32, name="s_ps",
                                         tag="s")
                        e = s2e.tile([128, 2, 512], FP32R, name="e", tag="e")
                        for ri, (fo, fl) in enumerate(FS_NQ):
                            nc.tensor.matmul(
                                s_ps[:p, ri, 0:fl],
                                kt[po:po + 64, pr, koff:koff + p],
                                qt[po:po + 64, pr, fo:fo + fl],
                                start=True, stop=True)
                            nc.scalar.activation(out=e[:p, ri, 0:fl],
                                                 in_=s_ps[:p, ri, 0:fl],
                                                 func=AF.Exp, scale=SCALE)
                            nc.tensor.matmul(
                                ctx[:, ri, 0:fl],
                                vpp[:p, j, h * 128:(h + 1) * 128],
                                e[:p, ri, 0:fl],
                                start=(jj == 0), stop=(jj == len(kjs) - 1))
                    rc = s2r.tile([128, 2, 512], FP32, name="rc", tag="rc")
                    nc.vector.reciprocal(rc[co:co + 64, :, :],
                                         ctx[do_:do_ + 64, :, :])
                    for ri, (fo, fl) in enumerate(FS_NQ):
                        if ki == 0:
                            nc.vector.tensor_mul(
                                ct[po:po + 64, pr, fo:fo + fl],
                                ctx[co:co + 64, ri, 0:fl],
                                rc[co:co + 64, ri, 0:fl])
                        else:
                            tm = s2t.tile([128, 2, 512], FP32R, name="tm",
                                          tag="tm")
                            nc.vector.tensor_mul(tm[co:co + 64, ri, 0:fl],
                                                 ctx[co:co + 64, ri, 0:fl],
                                                 rc[co:co + 64, ri, 0:fl])
                            nc.vector.tensor_add(
                                ct[po:po + 64, pr, fo:fo + fl],
                                ct[po:po + 64, pr, fo:fo + fl],
                                tm[co:co + 64, ri, 0:fl])
        if dbg:
            nc.sync.dma_start(out=dbg["ct"][:, :, :], in_=ct.bitcast(FP32))

        # ---------------- stage 2b: img self-attention weights ----------------
        with tc.tile_pool(name=f"s2b{b}", bufs=3) as s2b, \
             tc.tile_pool(name=f"s2bd{b}", bufs=3) as s2bd, \
             tc.tile_pool(name=f"ps_w{b}", bufs=3, space="PSUM") as ps_w:
            for h in range(H):
                po, pr = (h % 2) * 64, h // 2
                for (qo, p) in _chunks(NI):
                    sp2 = ps_w.tile([128, 2, 512], FP32, name="sp2", tag="sp2")
                    pii = s2b.tile([128, NI], FP32, name="pii", tag="pii")
                    den = s2bd.tile([128, 2], FP32, name="den", tag="den")
                    for ri, (fo, fl) in enumerate(FS_NI):
                        nc.tensor.matmul(
                            sp2[:p, ri, 0:fl],
                            qt[po:po + 64, pr, qo:qo + p],
                            kt[po:po + 64, pr, fo:fo + fl],
                            start=True, stop=True)
                        flv = min(fl, NI - fo)
                        nc.scalar.activation(out=pii[:p, fo:fo + flv],
                                             in_=sp2[:p, ri, 0:flv],
                                             func=AF.Exp, scale=SCALE,
                                             accum_out=den[:p, ri:ri + 1])
                    nc.vector.tensor_add(den[:p, 0:1], den[:p, 0:1],
                                         den[:p, 1:2])
                    nc.vector.reciprocal(den[:p, 0:1], den[:p, 0:1])
                    nc.vector.tensor_scalar_mul(pii[:p, :], pii[:p, :],
                                                den[:p, 0:1])
                    nc.sync.dma_start(out=attn_w[b, h, qo:qo + p, :],
                                      in_=pii[:p, :])

        # ---------------- stage 3: output projections ----------------
        with tc.tile_pool(name=f"s3w{b}", bufs=2) as s3w, \
             tc.tile_pool(name=f"s3o{b}", bufs=3) as s3o, \
             tc.tile_pool(name=f"ps_o{b}", bufs=2, space="PSUM") as ps_o:
            for mi, (sfx, n, qoff, nm) in enumerate(MODS):
                wt = s3w.tile([128, DC, D], FP32R, name="wt3", tag="wt3")
                nc.sync.dma_start(
                    out=wt, in_=W[f"Wo{sfx}"][:, :].rearrange(
                        "(c p) o -> p c o", p=128).bitcast(FP32R))
                for (qo, p) in _chunks(n):
                    po3 = ps_o.tile([128, 2, 512], FP32, name="po3", tag="po3")
                    osb = s3o.tile([128, D], FP32, name="osb", tag="osb")
                    for ri, (fo, fl) in enumerate(FS_D):
                        for dci in range(DC):
                            nc.tensor.matmul(
                                po3[:p, ri, 0:fl],
                                ct[:, dci, qoff + qo:qoff + qo + p],
                                wt[:, dci, fo:fo + fl],
                                start=(dci == 0), stop=False)
                        nc.tensor.matmul(
                            po3[:p, ri, 0:fl], ones_row[0:1, 0:p],
                            brow[("o", sfx)][0:1, fo:fo + fl],
                            start=False, stop=True)
                        nc.vector.tensor_copy(osb[:p, fo:fo + fl],
                                              po3[:p, ri, 0:fl])
                    nc.sync.dma_start(out=out_d[sfx][b, qo:qo + p, :],
                                      in_=osb[:p, :])


_NC_CACHE = None
_LAST_RESULTS = None


def kernel(**inputs):
    global _NC_CACHE, _LAST_RESULTS
    ins = {k: np.ascontiguousarray(np.asarray(v), dtype=np.float32)
           for k, v in inputs.items()}
    if _NC_CACHE is None:
        _NC_CACHE = build()
    nc = _NC_CACHE
    in_maps = []
    for c in range(NCORES):
        b0 = c * BPC
        m = {}
        for sfx, n, off, nm in MODS:
            m[nm] = ins[nm][b0:b0 + BPC]
        for nm in WNAMES:
            m[nm] = ins[nm]
            m["b" + nm[1:]] = ins["b" + nm[1:]]
        in_maps.append(m)
    trace = os.environ.get("KERNEL_TRACE") == "1"
    res = bass_utils.run_bass_kernel_spmd(nc, in_maps,
                                          core_ids=list(range(NCORES)),
                                          trace=trace)
    _LAST_RESULTS = res
    out_img = np.concatenate([res.results[c]["out_img"]
                              for c in range(NCORES)], axis=0)
    out_txt = np.concatenate([res.results[c]["out_txt"]
                              for c in range(NCORES)], axis=0)
    out_cln = np.concatenate([res.results[c]["out_cln"]
                              for c in range(NCORES)], axis=0)
    weights = np.concatenate([res.results[c]["attn_w"]
                              for c in range(NCORES)], axis=0)
    return out_img, out_txt, out_cln, weights


if __name__ == "__main__":
    rng = np.random.default_rng(0)
    fake = {}
    for sfx, n, off, nm in MODS:
        fake[nm] = rng.standard_normal((B, n, D)).astype(np.float32)
    for nm in WNAMES:
        fake[nm] = (rng.standard_normal((D, D)) * 0.02).astype(np.float32)
        fake["b" + nm[1:]] = np.zeros(D, np.float32)
    outs = kernel(**fake)
    for o in outs:
        print(o.shape, o.dtype, float(np.abs(o).mean()))
